# revision 1
# baseline (speedup 1.0000x reference)
"""Trainium2 Bass kernel for nn_CausalSelfAttention_15178414424258.

GQA sliding-window causal attention (HQ=16, HK=4, D=64, WINDOW=1024) with
value-embedding gating, rope + qk rms-norm, out-projection.

Sharding: tensor-parallel over the 4 kv-head groups x data-parallel over the
2 batches = 8 cores. Each core handles one batch b and one kv group g
(4 q heads, 1 k head, 1 v head), produces a partial out-projection
(its 256 channels of the attention output against the matching w_o columns);
the host sums the 4 partials per batch.

On-core dataflow (all matmuls fp32r; scores kept transposed [t_k, t_q] so
softmax denominators come free via a ones-column appended to V):
  A) qkv = x @ w_qkv_shard^T (+ gate logit col), gate/sigmoid, v += gate*ve,
     rope via [x1|x1]*[c|s] + [x2|x2]*[-s|c], rms-norm via Ln/Exp rsqrt,
     PE-transpose q/k to [d, t].
  B) per 512-query chunk and head-pair: S^T = k^T q in PSUM, exp on ACT
     (scale=1/8 folds the 1/sqrt(D)), mask/zero edge quarters on GPSIMD,
     PV accumulate with ones-augmented V giving aoT + denom row,
     reciprocal + partition-broadcast-DMA + normalize into aoT.
  C) out_partial = aoT^T @ w_oT_shard, DMA to DRAM.
"""
import sys

sys.path.insert(0, "/opt/trn_rl_repo")

from contextlib import ExitStack  # noqa: E402

import numpy as np  # noqa: E402

import concourse.bass as bass  # noqa: E402
import concourse.tile as tile  # noqa: E402
from concourse import bacc, mybir  # noqa: E402
from concourse.bass_utils import run_bass_kernel_spmd  # noqa: E402

F32 = mybir.dt.float32
F32R = mybir.dt.float32r
BF16 = mybir.dt.bfloat16
AF = mybir.ActivationFunctionType
ALU = mybir.AluOpType
AX = mybir.AxisListType

B, T, E = 2, 2048, 1024
HQ, HK, D = 16, 4, 64
WINDOW = 1024
GATE_CH = 12
RMS_EPS = 1e-8
G = HQ // HK          # 4 q heads per kv group
TB = T // 128         # 16 t-blocks
NC_ = 4               # 512-wide query chunks
KT = E // 128         # 8 k-tiles for the qkv matmul

_CACHE = {}


def _active_m(c):
    return range(max(0, 4 * c - 8), 4 * c + 4)


def _pin_act_tables(nc):
    """Keep Exp/Ln only in the combined set so insert_act_table_loads
    emits a single table load instead of thrashing between sets."""
    from concourse import hw_specs
    tabs = hw_specs.get_activation_tables(nc.m.arch)
    for name, s in tabs.items():
        if name != "natural_log_exp_and_others":
            s.discard(AF.Exp)
            s.discard(AF.Ln)


def build_program():
    nc = bacc.Bacc("TRN2", target_bir_lowering=False, debug=False, num_devices=8)
    _pin_act_tables(nc)

    xT = nc.declare_dram_parameter("xT", [E, T], F32R, isOutput=False)
    wqkvT = nc.declare_dram_parameter("wqkvT", [E, 386], F32R, isOutput=False)
    ve3 = nc.declare_dram_parameter("ve3", [T, D], F32, isOutput=False)
    ropeA = nc.declare_dram_parameter("ropeA", [T, D], F32, isOutput=False)
    ropeB = nc.declare_dram_parameter("ropeB", [T, D], F32, isOutput=False)
    woT = nc.declare_dram_parameter("woT", [G * D, E], F32R, isOutput=False)
    maskC = nc.declare_dram_parameter("maskC", [128, 128], BF16, isOutput=False)
    maskW = nc.declare_dram_parameter("maskW", [128, 128], BF16, isOutput=False)
    ident = nc.declare_dram_parameter("ident", [128, 128], F32R, isOutput=False)
    identb = nc.declare_dram_parameter("identb", [128, 128], BF16, isOutput=False)
    out = nc.declare_dram_parameter("out", [T, E], F32, isOutput=True)

    with tile.TileContext(nc) as tc, ExitStack() as ctx:
        P = lambda **kw: ctx.enter_context(tc.tile_pool(**kw))
        pers = P(name="pers", bufs=1)
        xp = P(name="xp", bufs=2)
        tmp = P(name="tmp", bufs=2)
        p2p = P(name="p2p", bufs=6)
        outs = P(name="outs", bufs=3)
        # PSUM budget (8 banks): tag "s" 2x[128,1024] (scores + qkv/outproj),
        # tags "a0"/"a1" 1x[128,1024] each (PV accumulators, hp-alternating,
        # reused for the phase-A transposes)
        ps = P(name="ps", bufs=1, space="PSUM")

        # ---- persistent SBUF ----
        wq_sb = [pers.tile([128, 386], F32R, tag=f"wq{k}", name=f"wq{k}") for k in range(KT)]
        wo_sb = [pers.tile([128, E], F32R, tag=f"wo{k}", name=f"wo{k}") for k in range(2)]
        ra_sb = pers.tile([128, TB, D], F32, tag="ra")
        rb_sb = pers.tile([128, TB, D], F32, tag="rb")
        ve_sb = pers.tile([128, TB, D], F32, tag="ve")
        mc_sb = pers.tile([128, 128], BF16, tag="mc")
        mw_sb = pers.tile([128, 128], BF16, tag="mw")
        v1a = pers.tile([128, TB, 128], F32R, tag="v1a")   # [v | 1 | 0...]
        v1b = pers.tile([128, TB, 128], F32R, tag="v1b")   # [0...| 1 | v]
        qt_sb = [pers.tile([128, T], F32R, tag=f"qt{p}", name=f"qt{p}") for p in range(2)]
        kt_sb = pers.tile([128, T], F32R, tag="kt")  # kT duplicated in both halves
        aot = [pers.tile([128, T], F32R, tag=f"aot{p}", name=f"aot{p}") for p in range(2)]

        wq_r = wqkvT.rearrange("(k p) f -> k p f", p=128)
        wo_r = woT.rearrange("(k p) f -> k p f", p=128)
        xT_r0 = xT.rearrange("(k p) t -> k p t", p=128)
        x_first = [xp.tile([128, 512], F32R, tag=f"x{k}", name=f"x{k}")
                   for k in range(KT)]
        for k in range(KT):
            nc.sync.dma_start(x_first[k][:], xT_r0[k, :, 0:512])
            nc.sync.dma_start(wq_sb[k][:], wq_r[k])
        nc.sync.dma_start(ra_sb[:], ropeA.rearrange("(tb p) d -> p tb d", p=128))
        nc.sync.dma_start(rb_sb[:], ropeB.rearrange("(tb p) d -> p tb d", p=128))
        nc.sync.dma_start(ve_sb[:], ve3.rearrange("(tb p) d -> p tb d", p=128))
        for k in range(2):
            nc.sync.dma_start(wo_sb[k][:], wo_r[k])
        nc.sync.dma_start(mc_sb[:], maskC[:])
        nc.sync.dma_start(mw_sb[:], maskW[:])

        # ones/zeros pattern of the augmented V copies
        nc.vector.memset(v1a[:].bitcast(F32), 0.0)
        nc.vector.memset(v1b[:].bitcast(F32), 0.0)
        for tb in range(TB):
            nc.vector.memset(v1a[:, tb, 64:65].bitcast(F32), 1.0)
            nc.vector.memset(v1b[:, tb, 63:64].bitcast(F32), 1.0)

        identity = pers.tile([128, 128], F32R, tag="ident")
        nc.sync.dma_start(identity[:], ident[:])
        identity_b = pers.tile([128, 128], BF16, tag="identb")
        nc.sync.dma_start(identity_b[:], identb[:])

        xT_r = xT.rearrange("(k p) t -> k p t", p=128)

        # ================= Phase A =================
        qn_kn = {}
        for tb in range(TB):
            c, r = divmod(tb, 4)
            if r == 0:
                if c == 0:
                    x_sb = x_first
                else:
                    x_sb = [xp.tile([128, 512], F32R, tag=f"x{k}",
                                    name=f"x{k}") for k in range(KT)]
                    for k in range(KT):
                        nc.sync.dma_start(x_sb[k][:],
                                          xT_r[k, :, c * 512:(c + 1) * 512])
            qkv_ps = ps.tile([128, 1024], F32, tag="s", name="qkv_ps",
                             bufs=2)[:, 0:512]
            for k in range(KT):
                nc.tensor.matmul(qkv_ps[:, 0:386],
                                 x_sb[k][:, r * 128:(r + 1) * 128],
                                 wq_sb[k][:], start=(k == 0), stop=(k == KT - 1))
            # PSUM -> SBUF once (ACT) so rope/v-gate can run on GPSIMD
            qkv = tmp.tile([128, 386], F32, tag="qkvs", bufs=3)
            nc.scalar.copy(qkv[:], qkv_ps[:, 0:386])

            # gate = sigmoid(logit) via 1/(1+exp(-x)); v = qkv_v + gate*ve3
            eg = tmp.tile([128, 1], F32, tag="eg")
            nc.scalar.activation(eg[:], qkv[:, 384:385], AF.Exp, scale=-1.0)
            gp = tmp.tile([128, 1], F32, tag="gp")
            nc.vector.tensor_scalar_add(gp[:], eg[:], 1.0)
            gi = tmp.tile([128, 1], F32, tag="gi")
            nc.vector.reciprocal_approx_fast(gi[:], gp[:])
            vt = tmp.tile([128, D], F32, tag="vt")
            nc.vector.tensor_scalar_mul(vt[:], ve_sb[:, tb], gi[:])
            nc.vector.tensor_add(v1a[:, tb, 0:64], qkv[:, 320:384], vt[:])
            nc.gpsimd.tensor_copy(v1b[:, tb, 64:128], v1a[:, tb, 0:64])

            # rope: out = [x1|x1]*[c|s] + [x2|x2]*[-s|c]
            def rope(dst, src_ap, nh, eng):
                x1 = src_ap[:, :, 0:32].unsqueeze(2).broadcast_to([128, nh, 2, 32])
                x2 = src_ap[:, :, 32:64].unsqueeze(2).broadcast_to([128, nh, 2, 32])
                rav = (ra_sb[:, tb].rearrange("p (two d) -> p two d", two=2)
                       .unsqueeze(1).broadcast_to([128, nh, 2, 32]))
                rbv = (rb_sb[:, tb].rearrange("p (two d) -> p two d", two=2)
                       .unsqueeze(1).broadcast_to([128, nh, 2, 32]))
                dv = dst[:].rearrange("p (h two d) -> p h two d", h=nh, two=2)
                t1 = tmp.tile([128, nh * 64], F32, tag=f"t1{nh}")
                t1v = t1[:].rearrange("p (h two d) -> p h two d", h=nh, two=2)
                eng.tensor_tensor(t1v, x1, rav, ALU.mult)
                eng.tensor_tensor(dv, x2, rbv, ALU.mult)
                eng.tensor_add(dst[:], dst[:], t1[:])

            qr = tmp.tile([128, G * D], F32, tag="qr")
            rope(qr, qkv[:, 0:256].rearrange("p (h d) -> p h d", h=G), G,
                 nc.gpsimd)
            kr = tmp.tile([128, D], F32, tag="kr")
            rope(kr, qkv[:, 256:320].rearrange("p (h d) -> p h d", h=1), 1,
                 nc.vector)

            # rms-norm scales: rsqrt(mean(x^2)+eps) = exp(-0.5*ln(m))
            sq = tmp.tile([128, D], F32, tag="sq")
            ss = tmp.tile([128, 8], F32, tag="ss")
            for h in range(G):
                nc.scalar.activation(sq[:], qr[:, h * 64:(h + 1) * 64],
                                     AF.Square, accum_out=ss[:, h:h + 1])
            nc.scalar.activation(sq[:], kr[:], AF.Square,
                                 accum_out=ss[:, 4:5])
            m5 = tmp.tile([128, 5], F32, tag="m5")
            nc.vector.tensor_scalar(m5[:], ss[:, 0:5], 1.0 / D, RMS_EPS,
                                    ALU.mult, ALU.add)
            ln5 = tmp.tile([128, 5], F32, tag="ln5")
            nc.scalar.activation(ln5[:], m5[:], AF.Ln)
            rs5 = tmp.tile([128, 5], F32, tag="rs5")
            nc.scalar.activation(rs5[:], ln5[:], AF.Exp, scale=-0.5)

            qn = tmp.tile([128, G * D], F32R, tag="qn", bufs=4)
            for h in range(G):
                nc.vector.tensor_scalar_mul(
                    qn[:, h * 64:(h + 1) * 64], qr[:, h * 64:(h + 1) * 64],
                    rs5[:, h:h + 1])
            kn = tmp.tile([128, D], F32R, tag="kn", bufs=4)
            nc.vector.tensor_scalar_mul(kn[:], kr[:], rs5[:, 4:5])

            # transposes run 2 iterations behind so PE never waits on the
            # rope/rms chain of the current block
            qn_kn[tb] = (qn, kn)
            for dtb in ([tb - 2] if tb >= 2 else []) + \
                       ([tb - 1, tb] if tb == TB - 1 else []):
                dqn, dkn = qn_kn.pop(dtb)
                for p in range(2):
                    tq = ps.tile([128, 1024], F32R, tag=("a0", "a1")[p],
                                 name="tq", bufs=1)[:, 0:128]
                    nc.tensor.transpose(tq[:], dqn[:, p * 128:(p + 1) * 128],
                                        identity[:])
                    nc.vector.tensor_copy(
                        qt_sb[p][:, dtb * 128:(dtb + 1) * 128], tq[:])
                tk = ps.tile([128, 1024], F32R, tag="a0",
                             name="tk", bufs=1)[0:64, 0:128]
                nc.tensor.transpose(tk[:], dkn[:], identity[:])
                nc.vector.tensor_copy(kt_sb[0:64, dtb * 128:(dtb + 1) * 128],
                                      tk[:])
                if dtb % 4 == 3:
                    nc.sync.dma_start(
                        kt_sb[64:128, (dtb - 3) * 128:(dtb + 1) * 128],
                        kt_sb[0:64, (dtb - 3) * 128:(dtb + 1) * 128])

        # ========== Phase B + C, interleaved per 512-query chunk ==========
        # Both head-pair streams advance m-by-m in lockstep so the ACT
        # engine (exp) stays saturated; out-projection for the finished
        # chunk is emitted immediately so its PSUM->SBUF copies and output
        # DMAs overlap the next chunk's attention.
        for c in range(NC_):
            ms = list(_active_m(c))
            pvs = [ps.tile([128, 1024], F32, tag=("a0", "a1")[hp],
                           name="pv", bufs=1) for hp in range(2)]
            # order blocks so a full-span m comes first: its PV matmul
            # (start=True) initializes the whole accumulator, letting every
            # later PV run trimmed to its active span without memsets.
            spans = {}
            for m in ms:
                deltas = [4 * c + qpos - m for qpos in range(4)]
                act_q = [q for q in range(4) if 0 <= deltas[q] <= 8]
                spans[m] = (act_q[0], act_q[-1] + 1, deltas)
            mf = next(m for m in ms if spans[m][0] == 0 and spans[m][1] == 4)
            ms_o = [mf] + [m for m in ms if m != mf]
            DEPTH = 2
            pending = {0: [], 1: []}  # hp -> [(p2, mi)] awaiting PV
            for mi in range(len(ms_o) + DEPTH):
                for hp in range(2):
                    if mi < len(ms_o):
                        m = ms_o[mi]
                        qs, qe, deltas = spans[m]
                        sqs, sqe = qs, qe
                        if sqe - sqs == 1:           # N=128 runs at 1/4 rate;
                            if sqs >= 1:             # widen to 256 (even, fast)
                                sqs -= 1
                            else:
                                sqe += 1
                        w = (sqe - sqs) * 128
                        s2 = ps.tile([128, 1024], F32, tag="s", name="s2",
                                     bufs=2)
                        for hl in range(2):
                            o = hl * 512 + sqs * 128
                            nc.tensor.matmul(
                                s2[:, o:o + w],
                                kt_sb[hl * 64:(hl + 1) * 64,
                                      m * 128:(m + 1) * 128],
                                qt_sb[hp][hl * 64:(hl + 1) * 64,
                                          c * 512 + sqs * 128:
                                          c * 512 + sqe * 128],
                                start=True, stop=False,
                                tile_position=(hl * 64, 0),
                                skip_group_check=True)
                            for qpos in range(qs, qe):
                                mt = (mc_sb if deltas[qpos] == 0 else
                                      mw_sb if deltas[qpos] == 8 else None)
                                if mt is None:
                                    continue
                                qo = hl * 512 + qpos * 128
                                nc.tensor.matmul(
                                    s2[:, qo:qo + 128], identity_b[:], mt[:],
                                    start=False, stop=False,
                                    skip_group_check=True)
                        p2 = p2p.tile([128, 1024], F32R)
                        p2v = p2[:].rearrange("p (h f) -> p h f", h=2)
                        s2v = s2[:].rearrange("p (h f) -> p h f", h=2)
                        nc.scalar.activation(
                            p2v[:, :, qs * 128:qe * 128],
                            s2v[:, :, qs * 128:qe * 128],
                            AF.Exp, scale=0.125)
                    if mi >= DEPTH:
                        prev_p2, pmi = pending[hp].pop(0)
                        pm = ms_o[pmi]
                        pqs, pqe, _ = spans[pm]
                        st = (pmi == 0)
                        sp_ = (pmi == len(ms_o) - 1)
                        if st:
                            pqs, pqe = 0, 4
                        pw = (pqe - pqs) * 128
                        for half in range(2):
                            o = half * 512 + pqs * 128
                            nc.tensor.matmul(
                                pvs[hp][:, o:o + pw],
                                (v1a, v1b)[half][:, pm],
                                prev_p2[:, o:o + pw],
                                start=st, stop=sp_, skip_group_check=True)
                    if mi < len(ms_o):
                        pending[hp].append((p2, mi))
            for hp in range(2):
                pv = pvs[hp]
                # denominators: reciprocal straight from PSUM rows 63/64,
                # then partition-broadcast via DMA
                ri = tmp.tile([128, 1024], F32, tag="ri")
                nc.vector.reciprocal_approx_fast(ri[:], pv[:, :])
                rb2 = outs.tile([128, 512], F32, tag="rb2")
                nc.sync.dma_start(
                    rb2[0:64, :],
                    ri[64:65, 0:512].unsqueeze(1).broadcast_to([1, 64, 512]))
                nc.sync.dma_start(
                    rb2[64:128, :],
                    ri[63:64, 512:1024].unsqueeze(1).broadcast_to([1, 64, 512]))
                nc.vector.tensor_tensor(
                    aot[hp][0:64, c * 512:(c + 1) * 512],
                    pv[0:64, 0:512], rb2[0:64, :], ALU.mult)
                nc.vector.tensor_tensor(
                    aot[hp][64:128, c * 512:(c + 1) * 512],
                    pv[64:128, 512:1024], rb2[64:128, :], ALU.mult)
            # out-projection for this chunk; stores batched 4 t-blocks
            # per DMA to cut HWDGE serialization
            for fc in range(2):
                ob4 = outs.tile([128, 4, 512], F32, tag=f"ob{fc}",
                                name=f"ob{fc}", bufs=2)
                for r in range(4):
                    tb = 4 * c + r
                    op = ps.tile([128, 1024], F32, tag=("a1", "a0")[fc],
                                 name="op", bufs=1)[:, 0:512]
                    for k in range(2):
                        nc.tensor.matmul(op[:],
                                         aot[k][:, tb * 128:(tb + 1) * 128],
                                         wo_sb[k][:, fc * 512:(fc + 1) * 512],
                                         start=(k == 0), stop=(k == 1))
                    nc.vector.tensor_copy(ob4[:, r], op[:])
                nc.sync.dma_start(
                    out.rearrange("(cc r p) e -> cc r p e", r=4, p=128)
                       [c, :, :, fc * 512:(fc + 1) * 512]
                       .transpose([1, 0, 2]),
                    ob4[:])

    nc.compile()
    return nc


def _prep_inputs(x, value_embeds, rope_cos, rope_sin, w_qkv, w_gate, w_o):
    cos = np.asarray(rope_cos, np.float32)
    sin = np.asarray(rope_sin, np.float32)
    ropeA = np.concatenate([cos, sin], axis=1)
    ropeB = np.concatenate([-sin, cos], axis=1)
    ii = np.arange(128)[:, None]
    jj = np.arange(128)[None, :]
    import ml_dtypes
    maskC = np.where(ii <= jj, 0.0, -1e30).astype(ml_dtypes.bfloat16)
    maskW = np.where(ii >= jj, 0.0, -1e30).astype(ml_dtypes.bfloat16)
    maps = []
    for core in range(8):
        b, g = divmod(core, 4)
        wq = w_qkv[g * G * D:(g + 1) * G * D]              # [256, E]
        wk = w_qkv[(HQ + g) * D:(HQ + g + 1) * D]          # [64, E]
        wv = w_qkv[(HQ + HK + g) * D:(HQ + HK + g + 1) * D]
        gate_col = np.zeros((2, E), np.float32)
        gate_col[0, :GATE_CH] = w_gate[g]
        wqkvT = np.ascontiguousarray(
            np.concatenate([wq, wk, wv, gate_col], axis=0).T)  # [E, 386]
        maps.append({
            "xT": np.ascontiguousarray(x[b].T),
            "wqkvT": wqkvT,
            "ve3": np.ascontiguousarray(
                3.0 * value_embeds[b, :, g * D:(g + 1) * D]),
            "ropeA": ropeA, "ropeB": ropeB,
            "woT": np.ascontiguousarray(w_o[:, g * G * D:(g + 1) * G * D].T),
            "maskC": maskC, "maskW": maskW,
            "ident": np.eye(128, dtype=np.float32),
            "identb": np.eye(128, dtype=ml_dtypes.bfloat16),
        })
    return maps


def kernel(x, value_embeds, rope_cos, rope_sin, w_qkv, w_gate, w_o,
           trace=False):
    if "nc" not in _CACHE:
        _CACHE["nc"] = build_program()
    nc = _CACHE["nc"]
    in_maps = _prep_inputs(x, value_embeds, rope_cos, rope_sin,
                           w_qkv, w_gate, w_o)
    res = run_bass_kernel_spmd(nc, in_maps, list(range(8)), trace=trace)
    _CACHE["last_exec_time_ns"] = res.exec_time_ns
    out = np.empty((B, T, E), np.float32)
    for b in range(B):
        out[b] = sum(res.results[4 * b + g]["out"] for g in range(4))
    return out



# revision 26
# speedup vs baseline: 1.2473x; 1.2473x over previous
"""Trainium2 Bass kernel for nn_CausalSelfAttention_15178414424258.

GQA sliding-window causal attention (HQ=16, HK=4, D=64, WINDOW=1024) with
value-embedding gating, rope + qk rms-norm, out-projection.

Sharding: tensor-parallel over the 4 kv-head groups x data-parallel over the
2 batches = 8 cores. Each core handles one batch b and one kv group g
(4 q heads, 1 k head, 1 v head), produces a partial out-projection
(its 256 channels of the attention output against the matching w_o columns);
the host sums the 4 partials per batch.

v2: all inputs bf16 (host-cast), PE matmuls bf16 (full rate at any width),
rope fused to 3 DVE ops/tb across all 5 heads, sum-of-squares on DVE
(tensor_mul + tensor_reduce), rms scales on GPSIMD, 4-in-1 PE transposes
(q-pair x2, k, k-dup) with one batched DVE copy into a unified qkt tensor,
softmax denominators broadcast via GPSIMD partition_broadcast, out-projection
DMA'd straight from PSUM to DRAM (f32, 2 t-blocks per store), and phase A
(qkv/rope/rms) interleaved into phase B's m-loop so ACT(exp)/PE(matmul)
overlap DVE/Pool work.
"""
import sys

sys.path.insert(0, "/opt/trn_rl_repo")

from contextlib import ExitStack  # noqa: E402

import numpy as np  # noqa: E402

import concourse.bass as bass  # noqa: E402
import concourse.tile as tile  # noqa: E402
from concourse import bacc, mybir  # noqa: E402
from concourse.bass_utils import run_bass_kernel_spmd  # noqa: E402

F32 = mybir.dt.float32
BF16 = mybir.dt.bfloat16
AF = mybir.ActivationFunctionType
ALU = mybir.AluOpType
AX = mybir.AxisListType

B, T, E = 2, 2048, 1024
HQ, HK, D = 16, 4, 64
WINDOW = 1024
GATE_CH = 12
RMS_EPS = 1e-8
G = HQ // HK          # 4 q heads per kv group
TB = T // 128         # 16 t-blocks
NC_ = 4               # 512-wide query chunks
KT = E // 128         # 8 k-tiles for the qkv matmul

_CACHE = {}


def _active_m(c):
    return range(max(0, 4 * c - 8), 4 * c + 4)


def _pin_act_tables(nc):
    """Keep Exp/Ln only in the combined set so insert_act_table_loads
    emits a single table load instead of thrashing between sets."""
    from concourse import hw_specs
    tabs = hw_specs.get_activation_tables(nc.m.arch)
    for name, s in tabs.items():
        if name != "natural_log_exp_and_others":
            s.discard(AF.Exp)
            s.discard(AF.Ln)


def build_program(dbg=False):
    nc = bacc.Bacc("TRN2", target_bir_lowering=False, debug=False, num_devices=8)
    _pin_act_tables(nc)
    if dbg:
        d_qkt = nc.declare_dram_parameter("d_qkt", [128, 3, T], BF16,
                                          isOutput=True)
        d_v1 = nc.declare_dram_parameter("d_v1", [128, TB, 128], BF16,
                                         isOutput=True)
        d_aot = nc.declare_dram_parameter("d_aot", [2, 128, T], BF16,
                                          isOutput=True)

    xT = nc.declare_dram_parameter("xT", [E, T], BF16, isOutput=False)
    wqkvT = nc.declare_dram_parameter("wqkvT", [E, 386], BF16, isOutput=False)
    ve3 = nc.declare_dram_parameter("ve3", [T, D], BF16, isOutput=False)
    ropeA = nc.declare_dram_parameter("ropeA", [T, D], BF16, isOutput=False)
    ropeB = nc.declare_dram_parameter("ropeB", [T, D], BF16, isOutput=False)
    woT = nc.declare_dram_parameter("woT", [G * D, E], BF16, isOutput=False)
    maskC = nc.declare_dram_parameter("maskC", [128, 128], BF16, isOutput=False)
    maskW = nc.declare_dram_parameter("maskW", [128, 128], BF16, isOutput=False)
    identb = nc.declare_dram_parameter("identb", [128, 128], BF16, isOutput=False)
    out = nc.declare_dram_parameter("out", [T, E], BF16, isOutput=True)

    with tile.TileContext(nc) as tc, ExitStack() as ctx:
        P = lambda **kw: ctx.enter_context(tc.tile_pool(**kw))
        pers = P(name="pers", bufs=1)
        xp = P(name="xp", bufs=2)
        tmp = P(name="tmp", bufs=2)
        p2p = P(name="p2p", bufs=6)
        outs = P(name="outs", bufs=3)
        # PSUM budget (8 banks): tag "s" 2x[128,1024] f32 (qkv + scores +
        # transpose staging), tags "a0"/"a1" 1x[128,1024] each (PV
        # accumulators per head-pair; reused for out-proj tiles)
        ps = P(name="ps", bufs=1, space="PSUM")

        # ---- persistent SBUF ----
        wq_sb = pers.tile([128, KT, 386], BF16, tag="wq")
        wo_sb = pers.tile([128, 2, E], BF16, tag="wo")
        ra_sb = pers.tile([128, TB, D], BF16, tag="ra")
        rb_sb = pers.tile([128, TB, D], BF16, tag="rb")
        ve_sb = pers.tile([128, TB, D], BF16, tag="ve")
        mc_sb = pers.tile([128, 128], BF16, tag="mc")
        mw_sb = pers.tile([128, 128], BF16, tag="mw")
        idb_sb = pers.tile([128, 128], BF16, tag="idb")
        # augmented V (shared by both hl halves): [1@0 | 0(1:64) | v(64:128)]
        # -> PV rows: Z at partition 0, ao dims at 64:128
        v1 = pers.tile([128, TB, 128], BF16, tag="v1")
        # qkt: [*, 0, t] = q-pair0^T, [*, 1, t] = q-pair1^T, [*, 2, t] = k^T
        # (k duplicated in both 64-row halves)
        qkt = pers.tile([128, 3, T], BF16, tag="qkt")
        aot = [pers.tile([128, T], BF16, tag=f"aot{p}", name=f"aot{p}")
               for p in range(2)]
        ssall = pers.tile([128, TB, 5], F32, tag="ss")
        rsall = pers.tile([128, TB, 5], F32, tag="rs")

        wq_r = wqkvT.rearrange("(k p) f -> p k f", p=128)
        wo_r = woT.rearrange("(k p) f -> p k f", p=128)
        xT_r = xT.rearrange("(k p) t -> p k t", p=128)
        out_r = out.rearrange("(x p) e -> x p e", p=128)

        x_sb = [xp.tile([128, KT, 512], BF16, tag=f"x{c}", name=f"x{c}",
                        bufs=1) for c in range(4)]
        nc.sync.dma_start(wq_sb[:], wq_r)
        nc.sync.dma_start(x_sb[0][:], xT_r[:, :, 0:512])
        nc.sync.dma_start(ra_sb[:], ropeA.rearrange("(tb p) d -> p tb d", p=128))
        nc.sync.dma_start(rb_sb[:], ropeB.rearrange("(tb p) d -> p tb d", p=128))
        nc.sync.dma_start(ve_sb[:], ve3.rearrange("(tb p) d -> p tb d", p=128))
        nc.sync.dma_start(mc_sb[:], maskC[:])
        nc.sync.dma_start(mw_sb[:], maskW[:])
        nc.sync.dma_start(idb_sb[:], identb[:])
        nc.sync.dma_start(x_sb[1][:], xT_r[:, :, 512:1024])
        nc.sync.dma_start(x_sb[2][:], xT_r[:, :, 1024:1536])
        nc.sync.dma_start(wo_sb[:], wo_r)
        nc.sync.dma_start(x_sb[3][:], xT_r[:, :, 1536:2048])

        # ones/zeros pattern of the augmented V (values filled per tb)
        nc.gpsimd.memset(v1[:, :, 1:64], 0.0)
        nc.vector.memset(v1[:, :, 0:1], 1.0)

        def phase_a(tb):
            """qkv matmul + gate/v + rope + sum-of-squares for one t-block."""
            c, r = divmod(tb, 4)
            qkv_ps = ps.tile([128, 1024], F32, tag="s", name="qkv_ps",
                             bufs=2)[:, 0:512]
            for k in range(KT):
                nc.tensor.matmul(qkv_ps[:, 0:386],
                                 x_sb[c][:, k, r * 128:(r + 1) * 128],
                                 wq_sb[:, k], start=(k == 0), stop=(k == KT - 1))
            # PSUM -> SBUF once (ACT, bf16) so rope/v-gate run off-PSUM
            qkv = tmp.tile([128, 385], BF16, tag="qkvs", bufs=3)
            nc.scalar.copy(qkv[:], qkv_ps[:, 0:385])
            # gate = sigmoid(logit): exp(-x) on ACT, add/recip on DVE
            eg = tmp.tile([128, 1], F32, tag="eg")
            nc.scalar.activation(eg[:], qkv_ps[:, 384:385], AF.Exp, scale=-1.0)
            gp = tmp.tile([128, 1], F32, tag="gp")
            nc.vector.tensor_scalar_add(gp[:], eg[:], 1.0)
            gi = tmp.tile([128, 1], F32, tag="gi")
            nc.vector.reciprocal_approx_fast(gi[:], gp[:])
            vt = tmp.tile([128, D], BF16, tag="vt")
            nc.vector.tensor_scalar_mul(vt[:], ve_sb[:, tb], gi[:])
            nc.vector.tensor_add(v1[:, tb, 64:128], qkv[:, 320:384], vt[:])

            # rope over all 5 heads (q0..q3, k) in 3 DVE ops:
            # out = [x1|x1]*[c|s] + [x2|x2]*[-s|c]
            qkn = tmp.tile([128, 320], BF16, tag="qkn", bufs=6)
            x1 = (qkv[:, 0:320].rearrange("p (h d) -> p h d", h=5)[:, :, 0:32]
                  .unsqueeze(2).broadcast_to([128, 5, 2, 32]))
            x2 = (qkv[:, 0:320].rearrange("p (h d) -> p h d", h=5)[:, :, 32:64]
                  .unsqueeze(2).broadcast_to([128, 5, 2, 32]))
            rav = (ra_sb[:, tb].rearrange("p (two d) -> p two d", two=2)
                   .unsqueeze(1).broadcast_to([128, 5, 2, 32]))
            rbv = (rb_sb[:, tb].rearrange("p (two d) -> p two d", two=2)
                   .unsqueeze(1).broadcast_to([128, 5, 2, 32]))
            dv = qkn[:].rearrange("p (h two d) -> p h two d", h=5, two=2)
            t1 = tmp.tile([128, 320], BF16, tag="t1")
            t1v = t1[:].rearrange("p (h two d) -> p h two d", h=5, two=2)
            nc.vector.tensor_tensor(t1v, x1, rav, ALU.mult)
            nc.vector.tensor_tensor(dv, x2, rbv, ALU.mult)
            nc.vector.tensor_add(qkn[:], qkn[:], t1[:])

            # sum of squares per head -> ssall[:, tb]
            sq = tmp.tile([128, 320], BF16, tag="sq")
            nc.vector.tensor_mul(sq[:], qkn[:], qkn[:])
            nc.vector.tensor_reduce(
                ssall[:, tb], sq[:].rearrange("p (h d) -> p h d", h=5),
                AX.X, ALU.add)
            return qkn

        def phase_a_rs(g):
            """rsqrt(mean+eps) for a finished 4-tb group (ACT ln/exp)."""
            m5 = tmp.tile([128, 20], F32, tag="m5")
            nc.vector.tensor_scalar(m5[:], ssall[:, 4 * g:4 * g + 4], 1.0 / D,
                                    RMS_EPS, ALU.mult, ALU.add)
            ln5 = tmp.tile([128, 20], F32, tag="ln5")
            nc.scalar.activation(ln5[:], m5[:], AF.Ln)
            nc.scalar.activation(rsall[:, 4 * g:4 * g + 4], ln5[:], AF.Exp,
                                 scale=-0.5)

        def phase_a_tp(tb, qkn):
            """rms scales (Pool) + 4-in-1 PE transpose + batched DVE copy."""
            qns = tmp.tile([128, 320], BF16, tag="qns", bufs=4)
            for h in range(5):
                nc.gpsimd.tensor_scalar_mul(
                    qns[:, h * 64:(h + 1) * 64], qkn[:, h * 64:(h + 1) * 64],
                    rsall[:, tb, h:h + 1])
            tp = ps.tile([128, 1024], F32, tag="s", name="tp",
                         bufs=2)[:].bitcast(BF16)
            nc.tensor.transpose(tp[0:128, 0:128], qns[:, 0:128], idb_sb[:])
            nc.tensor.transpose(tp[0:128, 128:256], qns[:, 128:256], idb_sb[:])
            nc.tensor.transpose(tp[0:64, 256:384], qns[:, 256:320], idb_sb[:])
            nc.tensor.transpose(tp[64:128, 256:384], qns[:, 256:320], idb_sb[:])
            nc.vector.tensor_copy(
                qkt[:, :, tb * 128:(tb + 1) * 128],
                tp[:, 0:384].rearrange("p (c t) -> p c t", c=3))

        def phase_b(c, a_tbs, a_qkn):
            """attention + out-projection for one 512-query chunk, with
            phase-A slices for other t-blocks interleaved into the m-loop."""
            ms = list(_active_m(c))
            pvs = [ps.tile([128, 1024], F32, tag=("a0", "a1")[hp],
                           name="pv", bufs=1) for hp in range(2)]
            # order blocks so a full-span m comes first: its PV matmul
            # (start=True) initializes the whole accumulator
            spans = {}
            for m in ms:
                deltas = [4 * c + qpos - m for qpos in range(4)]
                act_q = [q for q in range(4) if 0 <= deltas[q] <= 8]
                spans[m] = (act_q[0], act_q[-1] + 1, deltas)
            mf = next(m for m in ms if spans[m][0] == 0 and spans[m][1] == 4)
            ms_o = [mf] + [m for m in ms if m != mf]
            DEPTH = 2
            pending = {0: [], 1: []}  # hp -> [(p2, mi)] awaiting PV
            n_mi = len(ms_o) + DEPTH
            slice_at = {(n_mi * i) // len(a_tbs): i
                        for i in range(len(a_tbs))} if a_tbs else {}
            for mi in range(n_mi):
                # interleave phase-A slices evenly across the m-loop
                if mi in slice_at:
                    tb = a_tbs[slice_at[mi]]
                    if tb < TB:
                        a_qkn[tb] = phase_a(tb)
                        if tb % 4 == 3:
                            phase_a_rs(tb // 4)
                    ptb = tb - 4   # lagged scales+transposes (needs rs)
                    if 0 <= ptb < TB:
                        phase_a_tp(ptb, a_qkn.pop(ptb))
                for hp in range(2):
                    if mi < len(ms_o):
                        m = ms_o[mi]
                        qs, qe, deltas = spans[m]
                        w = (qe - qs) * 128
                        s2 = ps.tile([128, 1024], F32, tag="s", name="s2",
                                     bufs=2)
                        for hl in range(2):
                            o = hl * 512 + qs * 128
                            nc.tensor.matmul(
                                s2[:, o:o + w],
                                qkt[hl * 64:(hl + 1) * 64, 2,
                                    m * 128:(m + 1) * 128],
                                qkt[hl * 64:(hl + 1) * 64, hp,
                                    c * 512 + qs * 128:c * 512 + qe * 128],
                                start=True, stop=False,
                                tile_position=(hl * 64, 0),
                                skip_group_check=True)
                            for qpos in range(qs, qe):
                                mt = (mc_sb if deltas[qpos] == 0 else
                                      mw_sb if deltas[qpos] == 8 else None)
                                if mt is None:
                                    continue
                                qo = hl * 512 + qpos * 128
                                nc.tensor.matmul(
                                    s2[:, qo:qo + 128], idb_sb[:], mt[:],
                                    start=False, stop=False,
                                    skip_group_check=True)
                        p2 = p2p.tile([128, 1024], BF16)
                        p2v = p2[:].rearrange("p (h f) -> p h f", h=2)
                        s2v = s2[:].rearrange("p (h f) -> p h f", h=2)
                        nc.scalar.activation(
                            p2v[:, :, qs * 128:qe * 128],
                            s2v[:, :, qs * 128:qe * 128],
                            AF.Exp, scale=0.125)
                    if mi >= DEPTH:
                        prev_p2, pmi = pending[hp].pop(0)
                        pm = ms_o[pmi]
                        pqs, pqe, _ = spans[pm]
                        st = (pmi == 0)
                        sp_ = (pmi == len(ms_o) - 1)
                        if st:
                            pqs, pqe = 0, 4
                        pw = (pqe - pqs) * 128
                        for half in range(2):
                            o = half * 512 + pqs * 128
                            nc.tensor.matmul(
                                pvs[hp][:, o:o + pw],
                                v1[:, pm],
                                prev_p2[:, o:o + pw],
                                start=st, stop=sp_, skip_group_check=True)
                    if mi < len(ms_o):
                        pending[hp].append((p2, mi))
            for hp in range(2):
                pv = pvs[hp]
                # denominators: reciprocal of PSUM rows 63/64 on DVE, then
                # partition-broadcast on GPSIMD, normalize into aot
                riA = outs.tile([64, 512], F32, tag="riA")
                riB = outs.tile([64, 512], F32, tag="riB")
                nc.vector.reciprocal_approx_fast(riA[0:1, :], pv[0:1, 0:512])
                nc.vector.reciprocal_approx_fast(riB[0:1, :],
                                                 pv[0:1, 512:1024])
                rbA = outs.tile([64, 512], F32, tag="rbA")
                rbB = outs.tile([64, 512], F32, tag="rbB")
                nc.gpsimd.partition_broadcast(rbA[:], riA[0:1, :], channels=64)
                nc.gpsimd.partition_broadcast(rbB[:], riB[0:1, :], channels=64)
                nc.vector.tensor_tensor(
                    aot[hp][0:64, c * 512:(c + 1) * 512],
                    pv[64:128, 0:512], rbA[:], ALU.mult)
                nc.vector.tensor_tensor(
                    aot[hp][64:128, c * 512:(c + 1) * 512],
                    pv[64:128, 512:1024], rbB[:], ALU.mult)
            # out-projection: two t-blocks per PSUM tile, copied to SBUF as
            # bf16 (alternating ACT/DVE) and stored; host sums partials
            for fc in range(2):
                for rp in range(2):
                    op = ps.tile([128, 1024], F32, tag=("a1", "a0")[rp],
                                 name="op", bufs=1)
                    for r2 in range(2):
                        tb = 4 * c + 2 * rp + r2
                        for k in range(2):
                            nc.tensor.matmul(
                                op[:, r2 * 512:(r2 + 1) * 512],
                                aot[k][:, tb * 128:(tb + 1) * 128],
                                wo_sb[:, k, fc * 512:(fc + 1) * 512],
                                start=(k == 0), stop=(k == 1))
                    ob = outs.tile([128, 2, 512], BF16, tag=f"ob{fc}{rp}",
                                   name="ob", bufs=2)
                    if fc == 0:
                        nc.scalar.copy(ob[:], op[:].rearrange(
                            "p (r e) -> p r e", r=2))
                    else:
                        nc.vector.tensor_copy(ob[:], op[:].rearrange(
                            "p (r e) -> p r e", r=2))
                    nc.sync.dma_start(
                        out_r[4 * c + 2 * rp:4 * c + 2 * rp + 2, :,
                              fc * 512:(fc + 1) * 512].transpose([1, 0, 2]),
                        ob[:])

        # ================= schedule =================
        a_qkn = {}
        for tb in range(8):          # groups 0,1 up front
            a_qkn[tb] = phase_a(tb)
            if tb % 4 == 3:
                phase_a_rs(tb // 4)
            if tb >= 4:
                phase_a_tp(tb - 4, a_qkn.pop(tb - 4))
        # B(0) + A-group2, B(1) + A-group3, B(2) + leftover transposes, B(3)
        phase_b(0, [8, 9, 10, 11], a_qkn)
        phase_b(1, [12, 13, 14, 15], a_qkn)
        phase_b(2, [16, 17, 18, 19], a_qkn)   # >=TB: transpose-only
        phase_b(3, [], a_qkn)
        if dbg:
            nc.sync.dma_start(d_qkt[:], qkt[:])
            nc.sync.dma_start(d_v1[:], v1[:])
            for p in range(2):
                nc.sync.dma_start(d_aot[p], aot[p][:])

    nc.compile()
    return nc


def _prep_inputs(x, value_embeds, rope_cos, rope_sin, w_qkv, w_gate, w_o):
    import ml_dtypes
    bf = ml_dtypes.bfloat16
    cos = np.asarray(rope_cos, np.float32)
    sin = np.asarray(rope_sin, np.float32)
    ropeA = np.concatenate([cos, sin], axis=1).astype(bf)
    ropeB = np.concatenate([-sin, cos], axis=1).astype(bf)
    ii = np.arange(128)[:, None]
    jj = np.arange(128)[None, :]
    maskC = np.where(ii <= jj, 0.0, -1e30).astype(bf)
    maskW = np.where(ii >= jj, 0.0, -1e30).astype(bf)
    maps = []
    for core in range(8):
        b, g = divmod(core, 4)
        wq = w_qkv[g * G * D:(g + 1) * G * D]              # [256, E]
        wk = w_qkv[(HQ + g) * D:(HQ + g + 1) * D]          # [64, E]
        wv = w_qkv[(HQ + HK + g) * D:(HQ + HK + g + 1) * D]
        gate_col = np.zeros((2, E), np.float32)
        gate_col[0, :GATE_CH] = w_gate[g]
        wqkvT = np.ascontiguousarray(
            np.concatenate([wq, wk, wv, gate_col], axis=0).T).astype(bf)
        maps.append({
            "xT": np.ascontiguousarray(x[b].T).astype(bf),
            "wqkvT": wqkvT,
            "ve3": np.ascontiguousarray(
                3.0 * value_embeds[b, :, g * D:(g + 1) * D]).astype(bf),
            "ropeA": ropeA, "ropeB": ropeB,
            "woT": np.ascontiguousarray(
                w_o[:, g * G * D:(g + 1) * G * D].T).astype(bf),
            "maskC": maskC, "maskW": maskW,
            "identb": np.eye(128, dtype=bf),
        })
    return maps


def kernel(x, value_embeds, rope_cos, rope_sin, w_qkv, w_gate, w_o,
           trace=False):
    if "nc" not in _CACHE:
        _CACHE["nc"] = build_program()
    nc = _CACHE["nc"]
    in_maps = _prep_inputs(x, value_embeds, rope_cos, rope_sin,
                           w_qkv, w_gate, w_o)
    res = run_bass_kernel_spmd(nc, in_maps, list(range(8)), trace=trace)
    _CACHE["last_exec_time_ns"] = res.exec_time_ns
    out = np.empty((B, T, E), np.float32)
    for b in range(B):
        out[b] = sum(res.results[4 * b + g]["out"].astype(np.float32)
                     for g in range(4))
    return out


# revision 62
# speedup vs baseline: 1.2983x; 1.0409x over previous
"""Trainium2 Bass kernel for nn_CausalSelfAttention_15178414424258.

GQA sliding-window causal attention (HQ=16, HK=4, D=64, WINDOW=1024) with
value-embedding gating, rope + qk rms-norm, out-projection.

Sharding: tensor-parallel over the 4 kv-head groups x data-parallel over the
2 batches = 8 cores. Each core handles one batch b and one kv group g
(4 q heads, 1 k head, 1 v head), produces a partial out-projection
(its 256 channels of the attention output against the matching w_o columns);
the host sums the 4 partials per batch.

v2: all inputs bf16 (host-cast), PE matmuls bf16 (full rate at any width),
rope fused to 3 DVE ops/tb across all 5 heads, sum-of-squares on DVE
(tensor_mul + tensor_reduce), rms scales on GPSIMD, 4-in-1 PE transposes
(q-pair x2, k, k-dup) with one batched DVE copy into a unified qkt tensor,
softmax denominators broadcast via GPSIMD partition_broadcast, out-projection
DMA'd straight from PSUM to DRAM (f32, 2 t-blocks per store), and phase A
(qkv/rope/rms) interleaved into phase B's m-loop so ACT(exp)/PE(matmul)
overlap DVE/Pool work.
"""
import sys

sys.path.insert(0, "/opt/trn_rl_repo")

from contextlib import ExitStack  # noqa: E402

import numpy as np  # noqa: E402

import concourse.bass as bass  # noqa: E402
import concourse.tile as tile  # noqa: E402
from concourse import bacc, mybir  # noqa: E402
from concourse.bass_utils import run_bass_kernel_spmd  # noqa: E402

F32 = mybir.dt.float32
BF16 = mybir.dt.bfloat16
AF = mybir.ActivationFunctionType
ALU = mybir.AluOpType
AX = mybir.AxisListType

B, T, E = 2, 2048, 1024
HQ, HK, D = 16, 4, 64
WINDOW = 1024
GATE_CH = 12
RMS_EPS = 1e-8
G = HQ // HK          # 4 q heads per kv group
TB = T // 128         # 16 t-blocks
NC_ = 4               # 512-wide query chunks
KT = E // 128         # 8 k-tiles for the qkv matmul

_CACHE = {}


def _active_m(c):
    return range(max(0, 4 * c - 8), 4 * c + 4)


def _pin_act_tables(nc):
    """Empty every set except the combined Exp/Ln one so the chooser can
    only ever pick it: exactly one table load, at kernel start."""
    from concourse import hw_specs
    tabs = hw_specs.get_activation_tables(nc.m.arch)
    for name, s in tabs.items():
        if name != "natural_log_exp_and_others":
            s.clear()


def build_program(dbg=False):
    nc = bacc.Bacc("TRN2", target_bir_lowering=False, debug=False, num_devices=8)
    _pin_act_tables(nc)
    if dbg:
        d_qkt = nc.declare_dram_parameter("d_qkt", [128, 3, T], BF16,
                                          isOutput=True)
        d_v1 = nc.declare_dram_parameter("d_v1", [128, TB, 128], BF16,
                                         isOutput=True)
        d_aot = nc.declare_dram_parameter("d_aot", [2, 128, T], BF16,
                                          isOutput=True)

    xT = nc.declare_dram_parameter("xT", [E, T], BF16, isOutput=False)
    wqkvT = nc.declare_dram_parameter("wqkvT", [E, 386], BF16, isOutput=False)
    # aux rows: [ropeA(64) | ropeB(64) | 3*value_embeds(64) | pad(64)]
    # (padded to 512-byte rows so the DMA runs at full descriptor rate)
    aux = nc.declare_dram_parameter("aux", [T, 256], BF16, isOutput=False)
    woT = nc.declare_dram_parameter("woT", [G * D, E], BF16, isOutput=False)
    maskC = nc.declare_dram_parameter("maskC", [128, 128], BF16, isOutput=False)
    maskW = nc.declare_dram_parameter("maskW", [128, 128], BF16, isOutput=False)
    identb = nc.declare_dram_parameter("identb", [128, 128], BF16, isOutput=False)
    out = nc.declare_dram_parameter("out", [T, E], BF16, isOutput=True)

    with tile.TileContext(nc) as tc, ExitStack() as ctx:
        P = lambda **kw: ctx.enter_context(tc.tile_pool(**kw))
        pers = P(name="pers", bufs=1)
        xp = P(name="xp", bufs=2)
        tmp = P(name="tmp", bufs=2)
        p2p = P(name="p2p", bufs=6)
        outs = P(name="outs", bufs=3)
        # PSUM budget (8 banks): tag "s" 2x[128,1024] f32 (qkv + scores +
        # transpose staging), tags "a0"/"a1" 1x[128,1024] each (PV
        # accumulators per head-pair; reused for out-proj tiles)
        ps = P(name="ps", bufs=1, space="PSUM")

        # ---- persistent SBUF ----
        wq_sb = pers.tile([128, KT, 386], BF16, tag="wq")
        wo_sb = pers.tile([128, 2, E], BF16, tag="wo")
        aux_sb = pers.tile([128, TB, 256], BF16, tag="aux")
        mc_sb = pers.tile([128, 128], BF16, tag="mc")
        mw_sb = pers.tile([128, 128], BF16, tag="mw")
        idb_sb = pers.tile([128, 128], BF16, tag="idb")
        # augmented V (shared by both hl halves): [1@0 | 0(1:64) | v(64:128)]
        # -> PV rows: Z at partition 0, ao dims at 64:128
        v1 = pers.tile([128, TB, 128], BF16, tag="v1")
        # qkt: [*, 0, t] = q-pair0^T, [*, 1, t] = q-pair1^T, [*, 2, t] = k^T
        # (k duplicated in both 64-row halves)
        qkt = pers.tile([128, 3, T], BF16, tag="qkt")
        aot = [pers.tile([128, T], BF16, tag=f"aot{p}", name=f"aot{p}")
               for p in range(2)]
        ssall = pers.tile([128, TB, 5], F32, tag="ss")
        rsall = pers.tile([128, TB, 5], F32, tag="rs")
        rsk8 = pers.tile([128, TB], F32, tag="rsk8")

        wq_r = wqkvT.rearrange("(k p) f -> p k f", p=128)
        wo_r = woT.rearrange("(k p) f -> p k f", p=128)
        xT_r = xT.rearrange("(k p) t -> p k t", p=128)
        out_r = out.rearrange("(x p) e -> x p e", p=128)

        x_sb = [xp.tile([128, KT, 512], BF16, tag=f"x{c}", name=f"x{c}",
                        bufs=1) for c in range(4)]
        aux_r = aux.rearrange("(tb p) d -> p tb d", p=128)
        nc.sync.dma_start(wq_sb[:], wq_r)
        nc.sync.dma_start(x_sb[0][:], xT_r[:, :, 0:512])
        nc.sync.dma_start(aux_sb[:, 0:4], aux_r[:, 0:4])
        nc.sync.dma_start(aux_sb[:, 4:16], aux_r[:, 4:16])
        nc.sync.dma_start(mc_sb[:], maskC[:])
        nc.sync.dma_start(mw_sb[:], maskW[:])
        nc.sync.dma_start(idb_sb[:], identb[:])
        nc.sync.dma_start(x_sb[1][:], xT_r[:, :, 512:1024])
        nc.sync.dma_start(x_sb[2][:], xT_r[:, :, 1024:1536])
        nc.sync.dma_start(wo_sb[:], wo_r)
        nc.sync.dma_start(x_sb[3][:], xT_r[:, :, 1536:2048])

        # ones/zeros pattern of the augmented V (values filled per tb)
        nc.gpsimd.memset(v1[:, :, 1:64], 0.0)
        nc.vector.memset(v1[:, :, 0:1], 1.0)

        def phase_a(tb):
            """qkv matmul + gate/v + rope + sum-of-squares for one t-block."""
            c, r = divmod(tb, 4)
            qkv_ps = ps.tile([128, 1024], F32, tag="s", name="qkv_ps",
                             bufs=2)[:, 0:512]
            for k in range(KT):
                nc.tensor.matmul(qkv_ps[:, 0:385],
                                 x_sb[c][:, k, r * 128:(r + 1) * 128],
                                 wq_sb[:, k, 0:385],
                                 start=(k == 0), stop=(k == KT - 1))
            # PSUM -> SBUF once (ACT, bf16) so rope/v-gate run off-PSUM
            qkv = tmp.tile([128, 385], BF16, tag="qkvs", bufs=3)
            nc.scalar.copy(qkv[:], qkv_ps[:, 0:385])
            eg = tmp.tile([128, 1], F32, tag="eg")
            nc.scalar.activation(eg[:], qkv_ps[:, 384:385], AF.Exp, scale=-1.0)

            # rope over all 5 heads (q0..q3, k) in 3 DVE ops:
            # out = [x1|x1]*[c|s] + [x2|x2]*[-s|c]
            qkn = tmp.tile([128, 320], BF16, tag="qkn", bufs=6)
            x1 = (qkv[:, 0:320].rearrange("p (h d) -> p h d", h=5)[:, :, 0:32]
                  .unsqueeze(2).broadcast_to([128, 5, 2, 32]))
            x2 = (qkv[:, 0:320].rearrange("p (h d) -> p h d", h=5)[:, :, 32:64]
                  .unsqueeze(2).broadcast_to([128, 5, 2, 32]))
            rav = (aux_sb[:, tb, 0:64].rearrange("p (two d) -> p two d", two=2)
                   .unsqueeze(1).broadcast_to([128, 5, 2, 32]))
            rbv = (aux_sb[:, tb, 64:128]
                   .rearrange("p (two d) -> p two d", two=2)
                   .unsqueeze(1).broadcast_to([128, 5, 2, 32]))
            dv = qkn[:].rearrange("p (h two d) -> p h two d", h=5, two=2)
            t1 = tmp.tile([128, 320], BF16, tag="t1")
            t1v = t1[:].rearrange("p (h two d) -> p h two d", h=5, two=2)
            nc.vector.tensor_tensor(t1v, x1, rav, ALU.mult)
            nc.vector.tensor_tensor(dv, x2, rbv, ALU.mult)
            nc.vector.tensor_add(qkn[:], qkn[:], t1[:])

            # sum of squares per head -> ssall[:, tb] (rs-critical chain:
            # rope -> sq -> reduce; gate/v ops run on Pool so they don't
            # delay rs on DVE)
            sq = tmp.tile([128, 320], BF16, tag="sq")
            nc.vector.tensor_mul(sq[:], qkn[:], qkn[:])
            nc.vector.tensor_reduce(
                ssall[:, tb], sq[:].rearrange("p (h d) -> p h d", h=5),
                AX.X, ALU.add)

            # gate = sigmoid(logit): v1 <- qkv_v + 3*sigmoid(l)*ve
            gp = tmp.tile([128, 1], F32, tag="gp")
            nc.vector.tensor_scalar_add(gp[:], eg[:], 1.0)
            gi = tmp.tile([128, 1], F32, tag="gi")
            nc.vector.reciprocal_approx_fast(gi[:], gp[:])
            vt = tmp.tile([128, D], BF16, tag="vt")
            nc.vector.tensor_scalar_mul(vt[:], aux_sb[:, tb, 128:192], gi[:])
            nc.vector.tensor_add(v1[:, tb, 64:128], qkv[:, 320:384], vt[:])
            return qkn

        def phase_a_rs(tb):
            """rsqrt(mean+eps) for one t-block (ACT ln/exp); also 0.125*rs_k
            for folding k's rms into the exp scale."""
            m5 = tmp.tile([128, 5], F32, tag="m5")
            nc.vector.tensor_scalar(m5[:], ssall[:, tb], 1.0 / D,
                                    RMS_EPS, ALU.mult, ALU.add)
            ln5 = tmp.tile([128, 5], F32, tag="ln5")
            nc.scalar.activation(ln5[:], m5[:], AF.Ln)
            nc.scalar.activation(rsall[:, tb], ln5[:], AF.Exp, scale=-0.5)
            nc.vector.tensor_scalar_mul(rsk8[:, tb:tb + 1],
                                        rsall[:, tb, 4:5], 0.125)

        def phase_a_tp(tb, qkn):
            """q rms scales (Pool) + 4-in-1 PE transpose + batched DVE copy.
            k stays unscaled: its rms factor rides the exp scale."""
            qns = tmp.tile([128, 256], BF16, tag="qns", bufs=4)
            for h in range(4):
                nc.gpsimd.tensor_scalar_mul(
                    qns[:, h * 64:(h + 1) * 64], qkn[:, h * 64:(h + 1) * 64],
                    rsall[:, tb, h:h + 1])
            tp = ps.tile([128, 1024], F32, tag="s", name="tp",
                         bufs=2)[:].bitcast(BF16)
            nc.tensor.transpose(tp[0:128, 0:128], qns[:, 0:128], idb_sb[:])
            nc.tensor.transpose(tp[0:128, 128:256], qns[:, 128:256], idb_sb[:])
            nc.tensor.transpose(tp[0:64, 256:384], qkn[:, 256:320], idb_sb[:])
            nc.tensor.transpose(tp[64:128, 256:384], qkn[:, 256:320], idb_sb[:])
            nc.vector.tensor_copy(
                qkt[:, :, tb * 128:(tb + 1) * 128],
                tp[:, 0:384].rearrange("p (c t) -> p c t", c=3))

        def phase_b(c, inject):
            """attention for one 512-query chunk; `inject` is a list of
            closures (phase-A slices, transposes, deferred out-projections)
            spread evenly across the m-loop iterations."""
            ms = list(_active_m(c))
            pvs = [ps.tile([128, 1024], F32, tag=("a0", "a1")[hp],
                           name="pv", bufs=1) for hp in range(2)]
            # order blocks so a full-span m comes first: its PV matmul
            # (start=True) initializes the whole accumulator
            spans = {}
            for m in ms:
                deltas = [4 * c + qpos - m for qpos in range(4)]
                act_q = [q for q in range(4) if 0 <= deltas[q] <= 8]
                spans[m] = (act_q[0], act_q[-1] + 1, deltas)
            mf = next(m for m in ms if spans[m][0] == 0 and spans[m][1] == 4)
            ms_o = [mf] + [m for m in ms if m != mf]
            DEPTH = 2
            pending = {0: [], 1: []}  # hp -> [(p2, mi)] awaiting PV
            n_mi = len(ms_o) + DEPTH
            for mi in range(n_mi):
                for ii in range((len(inject) * mi) // n_mi,
                                (len(inject) * (mi + 1)) // n_mi):
                    inject[ii]()
                for hp in range(2):
                    if mi < len(ms_o):
                        m = ms_o[mi]
                        qs, qe, deltas = spans[m]
                        w = (qe - qs) * 128
                        s2 = ps.tile([128, 1024], F32, tag="s", name="s2",
                                     bufs=2)
                        for hl in range(2):
                            o = hl * 512 + qs * 128
                            nc.tensor.matmul(
                                s2[:, o:o + w],
                                qkt[hl * 64:(hl + 1) * 64, 2,
                                    m * 128:(m + 1) * 128],
                                qkt[hl * 64:(hl + 1) * 64, hp,
                                    c * 512 + qs * 128:c * 512 + qe * 128],
                                start=True, stop=False,
                                tile_position=(hl * 64, 0),
                                skip_group_check=True)
                            for qpos in range(qs, qe):
                                mt = (mc_sb if deltas[qpos] == 0 else
                                      mw_sb if deltas[qpos] == 8 else None)
                                if mt is None:
                                    continue
                                qo = hl * 512 + qpos * 128
                                nc.tensor.matmul(
                                    s2[:, qo:qo + 128], idb_sb[:], mt[:],
                                    start=False, stop=False,
                                    skip_group_check=True)
                        p2 = p2p.tile([128, 1024], BF16)
                        p2v = p2[:].rearrange("p (h f) -> p h f", h=2)
                        s2v = s2[:].rearrange("p (h f) -> p h f", h=2)
                        # scale = 0.125 * rs_k[t_k of block m] (k's rms-norm
                        # folded in as a per-partition activation scale)
                        nc.scalar.activation(
                            p2v[:, :, qs * 128:qe * 128],
                            s2v[:, :, qs * 128:qe * 128],
                            AF.Exp, scale=rsk8[:, m:m + 1])
                    if mi >= DEPTH and pending[hp]:
                        prev_p2, pmi = pending[hp].pop(0)
                        pm = ms_o[pmi]
                        pqs, pqe, _ = spans[pm]
                        st = (pmi == 0)
                        sp_ = (pmi == len(ms_o) - 1)
                        if st:
                            pqs, pqe = 0, 4
                        pw = (pqe - pqs) * 128
                        for half in range(2):
                            o = half * 512 + pqs * 128
                            nc.tensor.matmul(
                                pvs[hp][:, o:o + pw],
                                v1[:, pm],
                                prev_p2[:, o:o + pw],
                                start=st, stop=sp_, skip_group_check=True)
                        if sp_:
                            # softmax epilogue immediately after this hp's
                            # last PV: reciprocal (DVE) -> partition
                            # broadcast (Pool) -> normalize into aot, in
                            # 256-col halves so the out-projection of the
                            # first two t-blocks can start early
                            pv = pvs[hp]
                            riA = outs.tile([64, 512], F32, tag="riA")
                            riB = outs.tile([64, 512], F32, tag="riB")
                            nc.vector.reciprocal_approx_fast(
                                riA[0:1, :], pv[0:1, 0:512])
                            nc.vector.reciprocal_approx_fast(
                                riB[0:1, :], pv[0:1, 512:1024])
                            rbA = outs.tile([64, 512], F32, tag="rbA")
                            rbB = outs.tile([64, 512], F32, tag="rbB")
                            nc.gpsimd.partition_broadcast(
                                rbA[:], riA[0:1, :], channels=64)
                            nc.gpsimd.partition_broadcast(
                                rbB[:], riB[0:1, :], channels=64)
                            for ha in range(2):
                                s_ = slice(ha * 256, ha * 256 + 256)
                                cs = slice(c * 512 + ha * 256,
                                           c * 512 + ha * 256 + 256)
                                nc.vector.tensor_tensor(
                                    aot[hp][0:64, cs], pv[64:128, s_],
                                    rbA[:, s_], ALU.mult)
                                nc.vector.tensor_tensor(
                                    aot[hp][64:128, cs],
                                    pv[64:128, 512 + ha * 256:
                                       768 + ha * 256],
                                    rbB[:, s_], ALU.mult)
                    if mi < len(ms_o):
                        pending[hp].append((p2, mi))
        def make_op(c, fc, rp):
            """deferred out-projection tile for chunk c: two t-blocks per
            PSUM tile (tag s), copied to SBUF bf16 (ACT/DVE alternating)
            and stored; host sums partials."""
            def em():
                op = ps.tile([128, 1024], F32, tag="s", name="op", bufs=2)
                for r2 in range(2):
                    tb = 4 * c + 2 * rp + r2
                    for k in range(2):
                        nc.tensor.matmul(
                            op[:, r2 * 512:(r2 + 1) * 512],
                            aot[k][:, tb * 128:(tb + 1) * 128],
                            wo_sb[:, k, fc * 512:(fc + 1) * 512],
                            start=(k == 0), stop=(k == 1))
                ob = outs.tile([128, 2, 512], BF16, tag=f"ob{fc}{rp}",
                               name="ob", bufs=2)
                nc.vector.tensor_copy(ob[:], op[:].rearrange(
                    "p (r e) -> p r e", r=2))
                nc.sync.dma_start(
                    out_r[4 * c + 2 * rp:4 * c + 2 * rp + 2, :,
                          fc * 512:(fc + 1) * 512].transpose([1, 0, 2]),
                    ob[:])
            return em

        # ================= schedule =================
        # prologue: A-group 0 + its rms scales + transposes; then each B(c)
        # carries A-group c+1 (slices, rs, transposes) and chunk c-1's
        # deferred out-projection, spread across its m-loop.
        a_qkn = {}

        def mk_a(tb):
            def em():
                a_qkn[tb] = phase_a(tb)
                phase_a_rs(tb)
            return em

        def mk_tp(tb):
            return lambda: phase_a_tp(tb, a_qkn.pop(tb))

        # PE warmup: back-to-back junk matmuls on the first-loaded weights
        # keep the PE busy through its p-state ramp so the first real qkv
        # matmuls run at full clock instead of 0.65 GHz
        warm = ps.tile([128, 1024], F32, tag="s", name="warm", bufs=2)
        for i in range(8):
            nc.tensor.matmul(warm[:, 0:386], wq_sb[:, 0, 0:128],
                             wq_sb[:, 0], start=True, stop=True,
                             skip_group_check=True)

        # prologue: group 0 with lag-1 transposes
        for tb in range(4):
            mk_a(tb)()
            if tb >= 1:
                mk_tp(tb - 1)()
        mk_tp(3)()
        ops = {c: [make_op(c, fc, rp) for fc in range(2) for rp in range(2)]
               for c in range(NC_)}
        for c in range(NC_):
            inject = []
            opq = list(ops[c - 1]) if c > 0 else []
            if c + 1 < NC_:
                g = c + 1
                # A-slices first, transposes later (their rope/rms chains
                # have drained by then, so PE never head-blocks on them);
                # ops of the previous chunk fill the gaps.
                for i in range(4):
                    inject.append(mk_a(4 * g + i))
                    if opq:
                        inject.append(opq.pop(0))
                for i in range(4):
                    inject.append(mk_tp(4 * g + i))
                    if opq:
                        inject.append(opq.pop(0))
            inject += opq
            phase_b(c, inject)
        for em in ops[NC_ - 1]:
            em()
        if dbg:
            nc.sync.dma_start(d_qkt[:], qkt[:])
            nc.sync.dma_start(d_v1[:], v1[:])
            for p in range(2):
                nc.sync.dma_start(d_aot[p], aot[p][:])

    nc.compile()
    return nc


def _prep_inputs(x, value_embeds, rope_cos, rope_sin, w_qkv, w_gate, w_o):
    import ml_dtypes
    bf = ml_dtypes.bfloat16
    cos = np.asarray(rope_cos, np.float32)
    sin = np.asarray(rope_sin, np.float32)
    ropeA = np.concatenate([cos, sin], axis=1)
    ropeB = np.concatenate([-sin, cos], axis=1)
    ii = np.arange(128)[:, None]
    jj = np.arange(128)[None, :]
    maskC = np.where(ii <= jj, 0.0, -1e30).astype(bf)
    maskW = np.where(ii >= jj, 0.0, -1e30).astype(bf)
    maps = []
    for core in range(8):
        b, g = divmod(core, 4)
        wq = w_qkv[g * G * D:(g + 1) * G * D]              # [256, E]
        wk = w_qkv[(HQ + g) * D:(HQ + g + 1) * D]          # [64, E]
        wv = w_qkv[(HQ + HK + g) * D:(HQ + HK + g + 1) * D]
        gate_col = np.zeros((2, E), np.float32)
        gate_col[0, :GATE_CH] = w_gate[g]
        wqkvT = np.ascontiguousarray(
            np.concatenate([wq, wk, wv, gate_col], axis=0).T).astype(bf)
        aux = np.zeros((T, 256), np.float32)
        aux[:, 0:64] = ropeA
        aux[:, 64:128] = ropeB
        aux[:, 128:192] = 3.0 * value_embeds[b, :, g * D:(g + 1) * D]
        maps.append({
            "xT": np.ascontiguousarray(x[b].T).astype(bf),
            "wqkvT": wqkvT,
            "aux": aux.astype(bf),
            "woT": np.ascontiguousarray(
                w_o[:, g * G * D:(g + 1) * G * D].T).astype(bf),
            "maskC": maskC, "maskW": maskW,
            "identb": np.eye(128, dtype=bf),
        })
    return maps


def kernel(x, value_embeds, rope_cos, rope_sin, w_qkv, w_gate, w_o,
           trace=False):
    if "nc" not in _CACHE:
        _CACHE["nc"] = build_program()
    nc = _CACHE["nc"]
    in_maps = _prep_inputs(x, value_embeds, rope_cos, rope_sin,
                           w_qkv, w_gate, w_o)
    res = run_bass_kernel_spmd(nc, in_maps, list(range(8)), trace=trace)
    _CACHE["last_exec_time_ns"] = res.exec_time_ns
    out = np.empty((B, T, E), np.float32)
    for b in range(B):
        out[b] = sum(res.results[4 * b + g]["out"].astype(np.float32)
                     for g in range(4))
    return out


# revision 71
# speedup vs baseline: 1.3094x; 1.0086x over previous
"""Trainium2 Bass kernel for nn_CausalSelfAttention_15178414424258.

GQA sliding-window causal attention (HQ=16, HK=4, D=64, WINDOW=1024) with
value-embedding gating, rope + qk rms-norm, out-projection.

Sharding: tensor-parallel over the 4 kv-head groups x data-parallel over the
2 batches = 8 cores. Each core handles one batch b and one kv group g
(4 q heads, 1 k head, 1 v head), produces a partial out-projection
(its 256 channels of the attention output against the matching w_o columns);
the host sums the 4 partials per batch.

v2: all inputs bf16 (host-cast), PE matmuls bf16 (full rate at any width),
rope fused to 3 DVE ops/tb across all 5 heads, sum-of-squares on DVE
(tensor_mul + tensor_reduce), rms scales on GPSIMD, 4-in-1 PE transposes
(q-pair x2, k, k-dup) with one batched DVE copy into a unified qkt tensor,
softmax denominators broadcast via GPSIMD partition_broadcast, out-projection
DMA'd straight from PSUM to DRAM (f32, 2 t-blocks per store), and phase A
(qkv/rope/rms) interleaved into phase B's m-loop so ACT(exp)/PE(matmul)
overlap DVE/Pool work.
"""
import sys

sys.path.insert(0, "/opt/trn_rl_repo")

from contextlib import ExitStack  # noqa: E402

import numpy as np  # noqa: E402

import concourse.bass as bass  # noqa: E402
import concourse.tile as tile  # noqa: E402
from concourse import bacc, mybir  # noqa: E402
from concourse.bass_utils import run_bass_kernel_spmd  # noqa: E402

F32 = mybir.dt.float32
BF16 = mybir.dt.bfloat16
AF = mybir.ActivationFunctionType
ALU = mybir.AluOpType
AX = mybir.AxisListType

B, T, E = 2, 2048, 1024
HQ, HK, D = 16, 4, 64
WINDOW = 1024
GATE_CH = 12
RMS_EPS = 1e-8
G = HQ // HK          # 4 q heads per kv group
TB = T // 128         # 16 t-blocks
NC_ = 4               # 512-wide query chunks
KT = E // 128         # 8 k-tiles for the qkv matmul

_CACHE = {}
WARM = 6


def _active_m(c):
    return range(max(0, 4 * c - 8), 4 * c + 4)


def _pin_act_tables(nc):
    """Empty every set except the combined Exp/Ln one so the chooser can
    only ever pick it: exactly one table load, at kernel start."""
    from concourse import hw_specs
    tabs = hw_specs.get_activation_tables(nc.m.arch)
    for name, s in tabs.items():
        if name != "natural_log_exp_and_others":
            s.clear()


def build_program(dbg=False):
    nc = bacc.Bacc("TRN2", target_bir_lowering=False, debug=False, num_devices=8)
    _pin_act_tables(nc)
    if dbg:
        d_qkt = nc.declare_dram_parameter("d_qkt", [128, 3, T], BF16,
                                          isOutput=True)
        d_v1 = nc.declare_dram_parameter("d_v1", [128, TB, 128], BF16,
                                         isOutput=True)
        d_aot = nc.declare_dram_parameter("d_aot", [2, 128, T], BF16,
                                          isOutput=True)

    xT = nc.declare_dram_parameter("xT", [E, T], BF16, isOutput=False)
    wqkvT = nc.declare_dram_parameter("wqkvT", [E, 386], BF16, isOutput=False)
    # aux rows: [ropeA(64) | ropeB(64) | 3*value_embeds(64) | pad(64)]
    # (padded to 512-byte rows so the DMA runs at full descriptor rate)
    aux = nc.declare_dram_parameter("aux", [T, 256], BF16, isOutput=False)
    woT = nc.declare_dram_parameter("woT", [G * D, E], BF16, isOutput=False)
    maskC = nc.declare_dram_parameter("maskC", [128, 128], BF16, isOutput=False)
    maskW = nc.declare_dram_parameter("maskW", [128, 128], BF16, isOutput=False)
    identb = nc.declare_dram_parameter("identb", [128, 128], BF16, isOutput=False)
    out = nc.declare_dram_parameter("out", [T, E], BF16, isOutput=True)

    with tile.TileContext(nc) as tc, ExitStack() as ctx:
        P = lambda **kw: ctx.enter_context(tc.tile_pool(**kw))
        pers = P(name="pers", bufs=1)
        xp = P(name="xp", bufs=2)
        tmp = P(name="tmp", bufs=3)
        p2p = P(name="p2p", bufs=8)
        outs = P(name="outs", bufs=4)
        # PSUM budget (8 banks): tag "s" 2x[128,1024] f32 (qkv + scores +
        # transpose staging), tags "a0"/"a1" 1x[128,1024] each (PV
        # accumulators per head-pair; reused for out-proj tiles)
        ps = P(name="ps", bufs=1, space="PSUM")

        # ---- persistent SBUF ----
        wq_sb = pers.tile([128, KT, 386], BF16, tag="wq")
        wo_sb = pers.tile([128, 2, E], BF16, tag="wo")
        aux_sb = pers.tile([128, TB, 256], BF16, tag="aux")
        mc_sb = pers.tile([128, 128], BF16, tag="mc")
        mw_sb = pers.tile([128, 128], BF16, tag="mw")
        idb_sb = pers.tile([128, 128], BF16, tag="idb")
        # augmented V (shared by both hl halves): [1@0 | 0(1:64) | v(64:128)]
        # -> PV rows: Z at partition 0, ao dims at 64:128
        v1 = pers.tile([128, TB, 128], BF16, tag="v1")
        # qkt: [*, 0, t] = q-pair0^T, [*, 1, t] = q-pair1^T, [*, 2, t] = k^T
        # (k duplicated in both 64-row halves)
        qkt = pers.tile([128, 3, T], BF16, tag="qkt")
        aot = [pers.tile([128, T], BF16, tag=f"aot{p}", name=f"aot{p}")
               for p in range(2)]
        ssall = pers.tile([128, TB, 5], F32, tag="ss")
        rsall = pers.tile([128, TB, 5], F32, tag="rs")
        rsk8 = pers.tile([128, TB], F32, tag="rsk8")

        wq_r = wqkvT.rearrange("(k p) f -> p k f", p=128)
        wo_r = woT.rearrange("(k p) f -> p k f", p=128)
        xT_r = xT.rearrange("(k p) t -> p k t", p=128)
        out_r = out.rearrange("(x p) e -> x p e", p=128)

        x_sb = [xp.tile([128, KT, 512], BF16, tag=f"x{c}", name=f"x{c}",
                        bufs=1) for c in range(4)]
        aux_r = aux.rearrange("(tb p) d -> p tb d", p=128)
        nc.sync.dma_start(wq_sb[:], wq_r)
        nc.sync.dma_start(x_sb[0][:], xT_r[:, :, 0:512])
        nc.sync.dma_start(aux_sb[:, 0:4], aux_r[:, 0:4])
        nc.sync.dma_start(aux_sb[:, 4:16], aux_r[:, 4:16])
        nc.sync.dma_start(mc_sb[:], maskC[:])
        nc.sync.dma_start(mw_sb[:], maskW[:])
        nc.sync.dma_start(idb_sb[:], identb[:])
        nc.sync.dma_start(x_sb[1][:], xT_r[:, :, 512:1024])
        nc.sync.dma_start(x_sb[2][:], xT_r[:, :, 1024:1536])
        nc.sync.dma_start(wo_sb[:], wo_r)
        nc.sync.dma_start(x_sb[3][:], xT_r[:, :, 1536:2048])

        # ones/zeros pattern of the augmented V (values filled per tb)
        nc.gpsimd.memset(v1[:, :, 1:64], 0.0)
        nc.vector.memset(v1[:, :, 0:1], 1.0)

        def phase_a(tb):
            """qkv matmul + gate/v + rope + sum-of-squares for one t-block."""
            c, r = divmod(tb, 4)
            qkv_ps = ps.tile([128, 1024], F32, tag="s", name="qkv_ps",
                             bufs=2)[:, 0:512]
            for k in range(KT):
                nc.tensor.matmul(qkv_ps[:, 0:385],
                                 x_sb[c][:, k, r * 128:(r + 1) * 128],
                                 wq_sb[:, k, 0:385],
                                 start=(k == 0), stop=(k == KT - 1))
            # PSUM -> SBUF once (ACT, bf16) so rope/v-gate run off-PSUM
            qkv = tmp.tile([128, 385], BF16, tag="qkvs", bufs=4)
            nc.scalar.copy(qkv[:], qkv_ps[:, 0:385])
            eg = tmp.tile([128, 1], F32, tag="eg")
            nc.scalar.activation(eg[:], qkv_ps[:, 384:385], AF.Exp, scale=-1.0)

            # rope over all 5 heads (q0..q3, k) in 3 DVE ops:
            # out = [x1|x1]*[c|s] + [x2|x2]*[-s|c]
            qkn = tmp.tile([128, 320], BF16, tag="qkn", bufs=6)
            x1 = (qkv[:, 0:320].rearrange("p (h d) -> p h d", h=5)[:, :, 0:32]
                  .unsqueeze(2).broadcast_to([128, 5, 2, 32]))
            x2 = (qkv[:, 0:320].rearrange("p (h d) -> p h d", h=5)[:, :, 32:64]
                  .unsqueeze(2).broadcast_to([128, 5, 2, 32]))
            rav = (aux_sb[:, tb, 0:64].rearrange("p (two d) -> p two d", two=2)
                   .unsqueeze(1).broadcast_to([128, 5, 2, 32]))
            rbv = (aux_sb[:, tb, 64:128]
                   .rearrange("p (two d) -> p two d", two=2)
                   .unsqueeze(1).broadcast_to([128, 5, 2, 32]))
            dv = qkn[:].rearrange("p (h two d) -> p h two d", h=5, two=2)
            t1 = tmp.tile([128, 320], BF16, tag="t1")
            t1v = t1[:].rearrange("p (h two d) -> p h two d", h=5, two=2)
            nc.vector.tensor_tensor(t1v, x1, rav, ALU.mult)
            nc.vector.tensor_tensor(dv, x2, rbv, ALU.mult)
            nc.vector.tensor_add(qkn[:], qkn[:], t1[:])

            # sum of squares per head -> ssall[:, tb] (rs-critical chain:
            # rope -> sq -> reduce; gate/v ops run on Pool so they don't
            # delay rs on DVE)
            sq = tmp.tile([128, 320], BF16, tag="sq")
            nc.vector.tensor_mul(sq[:], qkn[:], qkn[:])
            nc.vector.tensor_reduce(
                ssall[:, tb], sq[:].rearrange("p (h d) -> p h d", h=5),
                AX.X, ALU.add)

            # gate = sigmoid(logit): v1 <- qkv_v + 3*sigmoid(l)*ve
            gp = tmp.tile([128, 1], F32, tag="gp")
            nc.vector.tensor_scalar_add(gp[:], eg[:], 1.0)
            gi = tmp.tile([128, 1], F32, tag="gi")
            nc.vector.reciprocal_approx_fast(gi[:], gp[:])
            vt = tmp.tile([128, D], BF16, tag="vt")
            nc.vector.tensor_scalar_mul(vt[:], aux_sb[:, tb, 128:192], gi[:])
            nc.vector.tensor_add(v1[:, tb, 64:128], qkv[:, 320:384], vt[:])
            return qkn

        def phase_a_rs(tb):
            """rsqrt(mean+eps) for one t-block (ACT ln/exp); also 0.125*rs_k
            for folding k's rms into the exp scale."""
            m5 = tmp.tile([128, 5], F32, tag="m5")
            nc.vector.tensor_scalar(m5[:], ssall[:, tb], 1.0 / D,
                                    RMS_EPS, ALU.mult, ALU.add)
            ln5 = tmp.tile([128, 5], F32, tag="ln5")
            nc.scalar.activation(ln5[:], m5[:], AF.Ln)
            nc.scalar.activation(rsall[:, tb], ln5[:], AF.Exp, scale=-0.5)
            nc.vector.tensor_scalar_mul(rsk8[:, tb:tb + 1],
                                        rsall[:, tb, 4:5], 0.125)

        def phase_a_tp(tb, qkn):
            """q rms scales (Pool) + 4-in-1 PE transpose + batched DVE copy.
            k stays unscaled: its rms factor rides the exp scale."""
            qns = tmp.tile([128, 256], BF16, tag="qns", bufs=6)
            for h in range(4):
                nc.gpsimd.tensor_scalar_mul(
                    qns[:, h * 64:(h + 1) * 64], qkn[:, h * 64:(h + 1) * 64],
                    rsall[:, tb, h:h + 1])
            tp = ps.tile([128, 1024], F32, tag="s", name="tp",
                         bufs=2)[:].bitcast(BF16)
            nc.tensor.transpose(tp[0:128, 0:128], qns[:, 0:128], idb_sb[:])
            nc.tensor.transpose(tp[0:128, 128:256], qns[:, 128:256], idb_sb[:])
            nc.tensor.transpose(tp[0:64, 256:384], qkn[:, 256:320], idb_sb[:])
            nc.tensor.transpose(tp[64:128, 256:384], qkn[:, 256:320], idb_sb[:])
            nc.vector.tensor_copy(
                qkt[:, :, tb * 128:(tb + 1) * 128],
                tp[:, 0:384].rearrange("p (c t) -> p c t", c=3))

        def phase_b(c, inject):
            """attention for one 512-query chunk; `inject` is a list of
            closures (phase-A slices, transposes, deferred out-projections)
            spread evenly across the m-loop iterations."""
            ms = list(_active_m(c))
            pvs = [ps.tile([128, 1024], F32, tag=("a0", "a1")[hp],
                           name="pv", bufs=1) for hp in range(2)]
            # order blocks so a full-span m comes first: its PV matmul
            # (start=True) initializes the whole accumulator
            spans = {}
            for m in ms:
                deltas = [4 * c + qpos - m for qpos in range(4)]
                act_q = [q for q in range(4) if 0 <= deltas[q] <= 8]
                spans[m] = (act_q[0], act_q[-1] + 1, deltas)
            mf = next(m for m in ms if spans[m][0] == 0 and spans[m][1] == 4)
            ms_o = [mf] + [m for m in ms if m != mf]
            DEPTH = 2
            pending = {0: [], 1: []}  # hp -> [(p2, mi)] awaiting PV
            n_mi = len(ms_o) + DEPTH
            for mi in range(n_mi):
                for ii in range((len(inject) * mi) // n_mi,
                                (len(inject) * (mi + 1)) // n_mi):
                    inject[ii]()
                for hp, act in [(0, "s"), (1, "s"), (0, "pv"), (1, "pv")]:
                    if act == "s" and mi < len(ms_o):
                        m = ms_o[mi]
                        qs, qe, deltas = spans[m]
                        w = (qe - qs) * 128
                        s2 = ps.tile([128, 1024], F32, tag="s", name="s2",
                                     bufs=2)
                        for hl in range(2):
                            o = hl * 512 + qs * 128
                            nc.tensor.matmul(
                                s2[:, o:o + w],
                                qkt[hl * 64:(hl + 1) * 64, 2,
                                    m * 128:(m + 1) * 128],
                                qkt[hl * 64:(hl + 1) * 64, hp,
                                    c * 512 + qs * 128:c * 512 + qe * 128],
                                start=True, stop=False,
                                tile_position=(hl * 64, 0),
                                skip_group_check=True)
                            for qpos in range(qs, qe):
                                mt = (mc_sb if deltas[qpos] == 0 else
                                      mw_sb if deltas[qpos] == 8 else None)
                                if mt is None:
                                    continue
                                qo = hl * 512 + qpos * 128
                                nc.tensor.matmul(
                                    s2[:, qo:qo + 128], idb_sb[:], mt[:],
                                    start=False, stop=False,
                                    skip_group_check=True)
                        p2 = p2p.tile([128, 1024], BF16)
                        p2v = p2[:].rearrange("p (h f) -> p h f", h=2)
                        s2v = s2[:].rearrange("p (h f) -> p h f", h=2)
                        # scale = 0.125 * rs_k[t_k of block m] (k's rms-norm
                        # folded in as a per-partition activation scale)
                        nc.scalar.activation(
                            p2v[:, :, qs * 128:qe * 128],
                            s2v[:, :, qs * 128:qe * 128],
                            AF.Exp, scale=rsk8[:, m:m + 1])
                    if act == "pv" and mi >= DEPTH and pending[hp]:
                        prev_p2, pmi = pending[hp].pop(0)
                        pm = ms_o[pmi]
                        pqs, pqe, _ = spans[pm]
                        st = (pmi == 0)
                        sp_ = (pmi == len(ms_o) - 1)
                        if st:
                            pqs, pqe = 0, 4
                        pw = (pqe - pqs) * 128
                        for half in range(2):
                            o = half * 512 + pqs * 128
                            nc.tensor.matmul(
                                pvs[hp][:, o:o + pw],
                                v1[:, pm],
                                prev_p2[:, o:o + pw],
                                start=st, stop=sp_, skip_group_check=True)
                        if sp_:
                            # softmax epilogue immediately after this hp's
                            # last PV: reciprocal (DVE) -> partition
                            # broadcast (Pool) -> normalize into aot, in
                            # 256-col halves so the out-projection of the
                            # first two t-blocks can start early
                            pv = pvs[hp]
                            riA = outs.tile([64, 512], F32, tag="riA")
                            riB = outs.tile([64, 512], F32, tag="riB")
                            nc.vector.reciprocal_approx_fast(
                                riA[0:1, :], pv[0:1, 0:512])
                            nc.vector.reciprocal_approx_fast(
                                riB[0:1, :], pv[0:1, 512:1024])
                            rbA = outs.tile([64, 512], F32, tag="rbA")
                            rbB = outs.tile([64, 512], F32, tag="rbB")
                            nc.gpsimd.partition_broadcast(
                                rbA[:], riA[0:1, :], channels=64)
                            nc.gpsimd.partition_broadcast(
                                rbB[:], riB[0:1, :], channels=64)
                            for ha in range(2):
                                s_ = slice(ha * 256, ha * 256 + 256)
                                cs = slice(c * 512 + ha * 256,
                                           c * 512 + ha * 256 + 256)
                                nc.vector.tensor_tensor(
                                    aot[hp][0:64, cs], pv[64:128, s_],
                                    rbA[:, s_], ALU.mult)
                                nc.vector.tensor_tensor(
                                    aot[hp][64:128, cs],
                                    pv[64:128, 512 + ha * 256:
                                       768 + ha * 256],
                                    rbB[:, s_], ALU.mult)
                    if act == "s" and mi < len(ms_o):
                        pending[hp].append((p2, mi))
        def make_op(c, fc, rp):
            """deferred out-projection tile for chunk c: two t-blocks per
            PSUM tile (tag s), copied to SBUF bf16 (ACT/DVE alternating)
            and stored; host sums partials."""
            def em():
                op = ps.tile([128, 1024], F32, tag="s", name="op", bufs=2)
                for r2 in range(2):
                    tb = 4 * c + 2 * rp + r2
                    for k in range(2):
                        nc.tensor.matmul(
                            op[:, r2 * 512:(r2 + 1) * 512],
                            aot[k][:, tb * 128:(tb + 1) * 128],
                            wo_sb[:, k, fc * 512:(fc + 1) * 512],
                            start=(k == 0), stop=(k == 1))
                ob = outs.tile([128, 2, 512], BF16, tag=f"ob{fc}{rp}",
                               name="ob", bufs=2)
                nc.vector.tensor_copy(ob[:], op[:].rearrange(
                    "p (r e) -> p r e", r=2))
                nc.sync.dma_start(
                    out_r[4 * c + 2 * rp:4 * c + 2 * rp + 2, :,
                          fc * 512:(fc + 1) * 512].transpose([1, 0, 2]),
                    ob[:])
            return em

        # ================= schedule =================
        # prologue: A-group 0 + its rms scales + transposes; then each B(c)
        # carries A-group c+1 (slices, rs, transposes) and chunk c-1's
        # deferred out-projection, spread across its m-loop.
        a_qkn = {}

        def mk_a(tb):
            def em():
                a_qkn[tb] = phase_a(tb)
                phase_a_rs(tb)
            return em

        def mk_tp(tb):
            return lambda: phase_a_tp(tb, a_qkn.pop(tb))

        # PE warmup: back-to-back junk matmuls on the first-loaded weights
        # keep the PE busy through its p-state ramp so the first real qkv
        # matmuls run at full clock instead of 0.65 GHz
        warm = ps.tile([128, 1024], F32, tag="s", name="warm", bufs=2)
        for i in range(WARM):
            nc.tensor.matmul(warm[:, 0:386], wq_sb[:, 0, 0:128],
                             wq_sb[:, 0], start=True, stop=True,
                             skip_group_check=True)

        # prologue: group 0 with lag-1 transposes
        for tb in range(4):
            mk_a(tb)()
            if tb >= 1:
                mk_tp(tb - 1)()
        mk_tp(3)()
        ops = {c: [make_op(c, fc, rp) for fc in range(2) for rp in range(2)]
               for c in range(NC_)}
        for c in range(NC_):
            inject = []
            opq = list(ops[c - 1]) if c > 0 else []
            if c + 1 < NC_:
                g = c + 1
                # A-slices first, transposes later (their rope/rms chains
                # have drained by then, so PE never head-blocks on them);
                # ops of the previous chunk fill the gaps.
                for i in range(4):
                    inject.append(mk_a(4 * g + i))
                    if opq:
                        inject.append(opq.pop(0))
                for i in range(4):
                    inject.append(mk_tp(4 * g + i))
                    if opq:
                        inject.append(opq.pop(0))
            inject += opq
            phase_b(c, inject)
        for em in ops[NC_ - 1]:
            em()
        if dbg:
            nc.sync.dma_start(d_qkt[:], qkt[:])
            nc.sync.dma_start(d_v1[:], v1[:])
            for p in range(2):
                nc.sync.dma_start(d_aot[p], aot[p][:])

    nc.compile()
    return nc


def _prep_inputs(x, value_embeds, rope_cos, rope_sin, w_qkv, w_gate, w_o):
    import ml_dtypes
    bf = ml_dtypes.bfloat16
    cos = np.asarray(rope_cos, np.float32)
    sin = np.asarray(rope_sin, np.float32)
    ropeA = np.concatenate([cos, sin], axis=1)
    ropeB = np.concatenate([-sin, cos], axis=1)
    ii = np.arange(128)[:, None]
    jj = np.arange(128)[None, :]
    maskC = np.where(ii <= jj, 0.0, -1e30).astype(bf)
    maskW = np.where(ii >= jj, 0.0, -1e30).astype(bf)
    maps = []
    for core in range(8):
        b, g = divmod(core, 4)
        wq = w_qkv[g * G * D:(g + 1) * G * D]              # [256, E]
        wk = w_qkv[(HQ + g) * D:(HQ + g + 1) * D]          # [64, E]
        wv = w_qkv[(HQ + HK + g) * D:(HQ + HK + g + 1) * D]
        gate_col = np.zeros((2, E), np.float32)
        gate_col[0, :GATE_CH] = w_gate[g]
        wqkvT = np.ascontiguousarray(
            np.concatenate([wq, wk, wv, gate_col], axis=0).T).astype(bf)
        aux = np.zeros((T, 256), np.float32)
        aux[:, 0:64] = ropeA
        aux[:, 64:128] = ropeB
        aux[:, 128:192] = 3.0 * value_embeds[b, :, g * D:(g + 1) * D]
        maps.append({
            "xT": np.ascontiguousarray(x[b].T).astype(bf),
            "wqkvT": wqkvT,
            "aux": aux.astype(bf),
            "woT": np.ascontiguousarray(
                w_o[:, g * G * D:(g + 1) * G * D].T).astype(bf),
            "maskC": maskC, "maskW": maskW,
            "identb": np.eye(128, dtype=bf),
        })
    return maps


def kernel(x, value_embeds, rope_cos, rope_sin, w_qkv, w_gate, w_o,
           trace=False):
    if "nc" not in _CACHE:
        _CACHE["nc"] = build_program()
    nc = _CACHE["nc"]
    in_maps = _prep_inputs(x, value_embeds, rope_cos, rope_sin,
                           w_qkv, w_gate, w_o)
    res = run_bass_kernel_spmd(nc, in_maps, list(range(8)), trace=trace)
    _CACHE["last_exec_time_ns"] = res.exec_time_ns
    out = np.empty((B, T, E), np.float32)
    for b in range(B):
        out[b] = sum(res.results[4 * b + g]["out"].astype(np.float32)
                     for g in range(4))
    return out
